# revision 44
# baseline (speedup 1.0000x reference)
"""Trainium2 Bass kernel for nn_CNF_76355928588411 (v2).

Data-parallel over N across 8 NeuronCores; t-hypernet on host. Per core:
32 windows of 1024 samples as [128, 512] dual-packed tiles (2 sample
groups x 64 ensembles on partitions).

Steady state per pair of windows (~1440ns):
  PE  : mm1 hp = blockdiag(W^T) @ x          (2x [128,512], fp16)
  ACT : t1 = tanh(hp + B)                    (one [128,1024] instr)
  DVE : t2[:, SQP:] = t1*t1                  (348 cols, 2x fp16)
  Pool: t2[:, :SQP] = t1*t1                  (676 cols)
  PE  : mm2 po = blockdiag(U/E)^T @ t1       (both windows, one iter)
  PE  : dl[32k:32k+2, 512h:] = wublk^T @ t2  (6-window [66,1024] group)
  DVE : ob = int8(po * S)                    (one [128,1024] instr)
  ACT : staging of a dl group every 6 windows (even iterations)
  DMA : ob pair -> dxh (int8); x batch fetch; gathered dl DMAs.

hp and po pair-tiles share one 3-slot psum pool (6 banks); the dl group
tile is [66,1024] (2 banks). dx ships as int8 with scale S = 127 /
max_d sum_e |U/E| (hard bound, no clipping); dl ships f32 through a
6-row partition-gather AP. Windows 0-1 ride in the cst DMA so the first
tanh starts ~1.3us earlier.
"""

import sys

sys.path.insert(0, "/opt/trn_rl_repo")

import numpy as np

import concourse.bass as bass
from concourse import bacc
import concourse.mybir as mybir
import concourse.tile as tile
from concourse.bass_utils import run_bass_kernel_spmd

F32 = mybir.dt.float32
F16 = mybir.dt.float16
I8 = mybir.dt.int8
AF = mybir.ActivationFunctionType

E, D, H_DIM, N = 64, 64, 512, 262144
BLOCK = E * D
NCORES = 8
NSH = N // NCORES          # 32768 samples per core
WIN = 1024                 # samples per window ([128, 512] dual-packed)
NWIN = NSH // WIN          # 32 windows
WQ = 4                     # windows per x DMA batch
NQ = NWIN // WQ            # 8 x batches
DLG = 3                    # windows per dl psum group tile
NDLG = (NWIN + DLG - 1) // DLG   # 11 dl groups (last has 2 windows)
SQP = 640                  # square cols (of 1024/pair) on Pool engine
MMSKEW = 3                 # pair p mm2s issue at it 2p+MMSKEW
SQSKEW = 3                 # pair p squares issue at it 2p+SQSKEW
EGSKEW = 4                 # pair p egress issues at it 2p+EGSKEW
DLSKEW = 8                 # dl-mm for window v at it v+DLSKEW (+1 if v%6==0)
NPAIR = NWIN // 2

_CACHED = {}


def _build_nc(scale_i8):
    nc = bacc.Bacc("TRN2", target_bir_lowering=False, debug=False,
                   num_devices=NCORES)
    xt = nc.dram_tensor("xt", [128, NSH // 2], F16, kind="ExternalInput")
    # cst cols: 0:128 Wblk, 128:256 UPblk, 256:258 wublk, 258:260 B (f32
    # bits), 260:1284 x windows 0-1
    cst = nc.dram_tensor("cst", [128, 1284], F16, kind="ExternalInput")
    dxh = nc.dram_tensor("dxh", [128, NSH // 2], I8, kind="ExternalOutput")
    dlh = nc.dram_tensor("dlh", [96, 512 * NDLG], F32, kind="ExternalOutput")

    with tile.TileContext(nc) as tc:
        with (
            tc.tile_pool(name="consts", bufs=1) as consts,
            tc.tile_pool(name="xin", bufs=6) as xin,
            tc.tile_pool(name="t1p", bufs=4) as t1p,
            tc.tile_pool(name="t2p", bufs=5) as t2p,
            tc.tile_pool(name="obp", bufs=5) as obp,
            tc.tile_pool(name="dlsp", bufs=1) as dlsp,
            tc.tile_pool(name="ps", bufs=3, space="PSUM") as ps,
            tc.tile_pool(name="ps_dl", bufs=2, space="PSUM") as ps_dl,
        ):
            cst_t = consts.tile([128, 1284], F16)
            dls = dlsp.tile([128, 512 * NDLG], F32)  # rows 0..65 used
            xqs = {}

            def fetch(q, cols=None, eng=None):
                # in-loop fetches ride the ACT queue after tanh: xin WAR
                # waits are ~0 (3 batches of prefetch slack)
                if q in xqs:
                    xq_t = xqs[q]
                else:
                    xq_t = xin.tile([128, WQ * 512], F16, tag="xq")
                    xqs[q] = xq_t
                lo = q * WQ * 512
                c0, c1 = cols if cols else (0, WQ * 512)
                (eng or nc.scalar).dma_start(out=xq_t[:, c0:c1],
                                             in_=xt[:, lo + c0:lo + c1])

            # preamble: cst (+ windows 0-1) first on ACT; batches 0..2 on
            # SP/DVE/SP so HWDGE pipelining starts immediately; warm the
            # ACT tanh table and the PE pstate clock meanwhile
            # split the cst DMA so the part mm1(0) needs lands first
            # critical piece on SP: its queue reaches the HWDGE first (the
            # ACT queue pays the act-table load + barrier first)
            nc.sync.dma_start(out=cst_t[:, 0:772], in_=cst[:, 0:772])
            nc.scalar.dma_start(out=cst_t[:, 772:1284], in_=cst[:, 772:1284])
            dummy = consts.tile([128, 2], F32)
            nc.vector.memset(dummy, 0.0)
            nc.scalar.activation(dummy[0:1, 1:2], dummy[0:1, 0:1], AF.Tanh)
            dummy16 = consts.tile([128, 2], F16)
            nc.vector.memset(dummy16, 0.0)
            dumb = consts.tile([128, 512], F16)
            nc.vector.memset(dumb, 0.0)
            warm_ps = ps.tile([128, 1024], F32, name="warm_ps", tag="pp")
            # PE pstate ramp: a tiny matmul at t=0 (costed at full clock),
            # then LOW-pstate 512-col dummies that keep the engine
            # continuously busy past the 3us ramp point while the x DMAs
            # land, so every real matmul is costed at full clock
            nc.tensor.matmul(warm_ps[0:2, 0:2], dummy16, dummy16[:, 0:2],
                             start=True, stop=True)
            for _ in range(5):
                nc.tensor.matmul(warm_ps[0:2, 0:512], dummy16, dumb,
                                 start=True, stop=True)
            # early batches: batch 1 split in half so windows 4,5 land
            # ~1.5us earlier (batch-1 data was gating pair 2's tanh)
            fetch(0, cols=(1024, 2048), eng=nc.sync)   # windows 2,3
            fetch(1, cols=(0, 1024), eng=nc.sync)      # windows 4,5
            fetch(1, cols=(1024, 2048), eng=nc.sync)   # windows 6,7
            fetch(2, cols=(0, 1024), eng=nc.sync)      # windows 8,9
            fetch(2, cols=(1024, 2048), eng=nc.sync)   # windows 10,11
            fetch(3, eng=nc.sync)

            wblk = cst_t[:, 0:128]
            upblk = cst_t[:, 128:256]
            wublk = cst_t[:, 256:258]
            bdup_t = cst_t[:, 258:260].bitcast(F32)

            hp_cur = {}    # pair index -> hp psum tile
            t1_cur = {}    # pair index -> t1 sbuf tile
            t2_cur = {}    # pair index -> t2 sbuf tile
            po_cur = {}    # pair index -> po psum tile
            dl_cur = {}    # current dl group psum tile

            def stage_m(p):
                # both mm2s of pair p in one iteration: the pair completes
                # early so next iteration's egress starts at its head
                po_cur[p] = ps.tile([128, 1024], F32, name="po", tag="pp")
                t1 = t1_cur[p]
                nc.tensor.matmul(po_cur[p][:, 0:512], upblk, t1[:, 0:512],
                                 start=True, stop=True)
                nc.tensor.matmul(po_cur[p][:, 512:1024], upblk, t1[:, 512:1024],
                                 start=True, stop=True)

            def stage_sq(p):
                # squares for pair p, one pair after its tanh: zero waits.
                # Last pair goes all-DVE so the drain skips the slow Pool op.
                t1 = t1_cur[p]
                t2 = t2p.tile([128, 1024], F16, name="t2")
                t2_cur[p] = t2
                if p == NPAIR - 1:
                    nc.vector.tensor_mul(t2, t1, t1)
                else:
                    nc.vector.tensor_mul(t2[:, SQP:1024], t1[:, SQP:1024],
                                         t1[:, SQP:1024])
                    nc.gpsimd.tensor_mul(t2[:, 0:SQP], t1[:, 0:SQP],
                                         t1[:, 0:SQP])

            def stage_e(p):
                # int8 egress + dx DMA for pair p; the last pair egresses on
                # ACT (idle after the last tanh) so the drain's DVE chain
                # (egress(14) + sq(15) + ...) doesn't serialize behind it
                po = po_cur.pop(p)
                ob = obp.tile([128, 1024], I8, name="ob")
                if p == NPAIR - 1:
                    nc.scalar.activation(ob, po, AF.Copy,
                                         scale=float(scale_i8))
                    nc.sync.dma_start(
                        out=dxh[:, 2 * p * 512:(2 * p + 2) * 512], in_=ob)
                else:
                    nc.vector.tensor_scalar_mul(ob, po, float(scale_i8))
                    nc.sync.dma_start(
                        out=dxh[:, 2 * p * 512:(2 * p + 2) * 512], in_=ob)

            def stage_dl(v):
                # dl matmul for window v (group g = v//3 at partition base
                # 32k of a double-buffered [66,512] tile)
                g, k = v // DLG, v % DLG
                if k == 0:
                    dl_cur[1] = dl_cur.get(0)
                    dl_cur[0] = ps_dl.tile([66, 512], F32, name="dl")
                nc.tensor.matmul(
                    dl_cur[0][32 * k:32 * k + 2, :],
                    wublk, t2_cur[v // 2][:, (v % 2) * 512:(v % 2) * 512 + 512],
                    start=True, stop=True)
                if v % 2 == 1:
                    del t1_cur[v // 2]
                    del t2_cur[v // 2]

            def stage_dl_post(v):
                # staging copy for the group completed one window ago, after
                # the iteration's tanh so the ACT never waits on PE dl-mms
                g, k = v // DLG, v % DLG
                if k == 0 and g > 0:
                    nc.scalar.copy(
                        dls[0:66, (g - 1) * 512:g * 512], dl_cur[1])
                    if g == NDLG - 2:
                        # groups 0..g-1 staged (the DMA waits on the copy
                        # just issued): ship early so the tail DMA is tiny
                        for s in range(2):
                            src = dls.rearrange("(a q) c -> a q c", a=4)[0:3, s]
                            dst = dlh.rearrange("(a q) c -> a q c", a=3)[:, s]
                            nc.sync.dma_start(out=dst[:, 0:g * 512],
                                              in_=src[:, 0:g * 512])
                if v == NWIN - 1:
                    # last group staged on DVE (ACT is busy with the final
                    # egress); ONE garbage-tolerant tail DMA (rows 0..65, the
                    # host ignores the unused rows) on the now-free ACT queue
                    nc.vector.tensor_copy(dls[0:66, g * 512:(g + 1) * 512],
                                          dl_cur[0])
                    nc.scalar.dma_start(
                        out=dlh[0:66, (g - 1) * 512:(g + 1) * 512],
                        in_=dls[0:66, (g - 1) * 512:(g + 1) * 512])

            def stage_a_mm(w):
                # mm1 for window w: issued FIRST on PE each iteration --
                # it has no upstream deps and feeds the critical tanh chain
                if w < 2:
                    xw = cst_t[:, 260 + w * 512:260 + (w + 1) * 512]
                else:
                    xq = xqs[w // WQ]
                    xw = xq[:, (w % WQ) * 512:(w % WQ + 1) * 512]
                if w % 2 == 0:
                    hp_cur[w // 2] = ps.tile([128, 1024], F32, name="hp",
                                             tag="pp")
                hp = hp_cur[w // 2]
                half = (w % 2) * 512
                nc.tensor.matmul(hp[:, half:half + 512], wblk, xw,
                                 start=True, stop=True)

            def stage_a_act(w):
                # tanh per pair; x prefetch
                if w % 2 == 1:
                    p = w // 2
                    t1 = t1p.tile([128, 1024], F16, name="t1")
                    t1_cur[p] = t1
                    nc.scalar.activation(t1, hp_cur[p], AF.Tanh,
                                         bias=bdup_t, scale=1.0)
                    del hp_cur[p]
                if w % WQ == 1 and w // WQ + 4 < NQ:
                    fetch(w // WQ + 4)
                if w % WQ == 3 and w // WQ in xqs:
                    del xqs[w // WQ]

            for it in range(NWIN + DLSKEW + 2):
                if it < NWIN:
                    stage_a_mm(it)
                if it >= MMSKEW and (it - MMSKEW) % 2 == 0 \
                        and (it - MMSKEW) // 2 < NPAIR:
                    stage_m((it - MMSKEW) // 2)
                sqit = it - SQSKEW
                if sqit >= 0 and sqit % 2 == 0 and sqit // 2 < NPAIR - 1:
                    stage_sq(sqit // 2)
                elif sqit == 2 * (NPAIR - 1) - 1:
                    # last pair's square one iteration early: it lands BEFORE
                    # egress(14) in DVE order, so the dl tail chain (sq ->
                    # dl-mms -> staging -> DMA) starts ~1.2us sooner;
                    # egress(14)'s dx DMA has slack to absorb the swap
                    stage_sq(NPAIR - 1)
                if it >= EGSKEW and (it - EGSKEW) % 2 == 0 \
                        and (it - EGSKEW) // 2 < NPAIR:
                    stage_e((it - EGSKEW) // 2)
                v3 = it - DLSKEW
                if 0 <= v3 < NWIN:
                    stage_dl(v3)
                if it < NWIN:
                    stage_a_act(it)
                if 0 <= v3 < NWIN:
                    stage_dl_post(v3)
    nc.compile()
    return nc


def _hypernet(t, W1, b1, W2, b2, W3, b3):
    p = np.tanh(t.reshape(1, 1) @ W1 + b1)
    p = np.tanh(p @ W2 + b2)
    p = (p @ W3 + b3).reshape(-1).astype(np.float32)
    W = p[:BLOCK].reshape(E, D)
    U = p[BLOCK:2 * BLOCK].reshape(E, D)
    G = 1.0 / (1.0 + np.exp(-p[2 * BLOCK:3 * BLOCK].reshape(E, D)))
    U = (U * G).astype(np.float32)
    B = p[3 * BLOCK:].reshape(E, 1).astype(np.float32)
    return W.astype(np.float32), U, B


def kernel(t, x, W1, b1, W2, b2, W3, b3):
    W, U, B = _hypernet(
        np.asarray(t, np.float32), np.asarray(W1, np.float32),
        np.asarray(b1, np.float32), np.asarray(W2, np.float32),
        np.asarray(b2, np.float32), np.asarray(W3, np.float32),
        np.asarray(b3, np.float32),
    )
    up16 = (U / E).astype(np.float16)
    wu = np.sum(W * U, axis=1).astype(np.float16)     # [E] (device dtype)

    bound = np.abs(up16.astype(np.float32)).sum(axis=0).max()
    scale_i8 = 127.0 / float(bound)

    # x [N, D] -> per-core [128, NSH//2] fp16; sample (c, w, s, j) at
    # partition s*64+d, column w*512+j
    xs = np.asarray(x, np.float16).reshape(NCORES, NWIN, 2, 512, D)
    xs = np.ascontiguousarray(xs.transpose(0, 2, 4, 1, 3))
    xl = xs.reshape(NCORES, 128, NSH // 2)

    cst = np.zeros((128, 258), np.float32)
    cst[0:64, 0:64] = W.T
    cst[64:128, 64:128] = W.T
    cst[0:64, 128:192] = up16.astype(np.float32)
    cst[64:128, 192:256] = up16.astype(np.float32)
    cst[0:64, 256] = wu.astype(np.float32)
    cst[64:128, 257] = wu.astype(np.float32)
    cst = cst.astype(np.float16)
    bdup = np.concatenate([B, B], axis=0).reshape(128, 1).astype(np.float32)
    cst = np.concatenate([cst, bdup.view(np.float16).reshape(128, 2)], axis=1)
    # append windows 0,1 per core
    csts = [np.concatenate([cst, xl[c][:, 0:1024]], axis=1)
            for c in range(NCORES)]

    if "nc" not in _CACHED:
        _CACHED["nc"] = _build_nc(scale_i8)
    nc = _CACHED["nc"]

    in_maps = [
        {"xt": xl[c], "cst": csts[c]}
        for c in range(NCORES)
    ]
    res = run_bass_kernel_spmd(nc, in_maps, core_ids=list(range(NCORES)))

    out = np.empty((N, D + 1), np.float32)
    od = out[:, :D].reshape(NCORES, NWIN, 2, 512, D)
    ol = out[:, D].reshape(NCORES, NWIN, 2, 512)
    sw = float(np.sum(wu.astype(np.float32)))
    inv_s = 1.0 / scale_i8
    for c in range(NCORES):
        dxc = res.results[c]["dxh"].astype(np.float32) * inv_s
        od[c] = dxc.reshape(2, D, NWIN, 512).transpose(2, 0, 3, 1)
        dlc = res.results[c]["dlh"].astype(np.float32)
        # dlh[32k+s, g*512+j] is window w = g*DLG + k, group s, sample j
        dlw = dlc.reshape(3, 32, NDLG, 512)[:, 0:2].transpose(2, 0, 1, 3)
        ol[c] = (dlw.reshape(NDLG * DLG, 2, 512)[:NWIN] - sw) / E
    return out
